# revision 40
# baseline (speedup 1.0000x reference)
"""Trainium2 Bass kernel for nn_AttentionBlock_15470472200943.

Causal multi-head attention block (B=8, T=1024, E=1024, H=16, D=64),
data-parallel: one batch element per NeuronCore across 8 cores.
~195us HW exec (profiled), down from the 205us prior best / 277us
original baseline.

Key design points:
- RoPE skipped: the module applies the identical rotation R to q and k
  at every position and R R^T = I cancels inside q @ k^T.
- Everything on the PE is fp16 (NOT bf16): same 1 col/cycle rate,
  3 more mantissa bits (rel_l2 5.5e-4 vs 3.9e-3), and it matches the
  fp16 score matmuls.  Output is stored fp16 and upconverted on host.
- Scores: fp16 K=64 matmuls on PE row groups 0/64 -- the two heads of
  a pair launch ~3ns apart and execute CONCURRENTLY on the array
  halves, halving score cost.  Each K=128<->K=64 mode edge exposes a
  ~100ns LDWEIGHTS (~1.9us/pair); attempts to remove it all fail:
  K=64-paired attn@v crashes the HW (two row-group streams writing
  one PSUM bank), zero-padded K=128 scores cost more than the tax,
  and emission-order grouping is ignored (the Tile scheduler is a
  ready-list scheduler, not in-order).
- Scores/exp/attn@v restricted to causally-live columns at 128-col
  granularity; only diagonal tiles get an elementwise tri-mask (DVE).
- No bias-via-matmul: qk bias folds into the DVE PSUM-evacuation,
  v bias is a 128-replicated SBUF tile added during the v evac, and
  the out-projection bias is added on the host.
- Softmax denominator comes out of the attn@v matmul itself
  (stationary [ones(64) | v_h(64)]); no max-subtraction (scores
  bounded, exp safe); 1/sqrt(D) folds into the exp scale.
- Engine placement: PE matmuls only; ACT does exp ONLY during
  attention (one joint exp per key-tile); DVE does PSUM evacs + tri
  masks + reciprocal + normalize.  The three are balanced within
  ~15% of each other in the pair loop -- moving work between them
  (tri->gpsimd, exp-split, sc->SBUF copies) makes some engine the
  new bottleneck.
- Attention emits scores in 2-jt groups with attn@v drained one full
  group later (sc PSUM pool = exactly 2 tiles), hiding exp latency
  behind next-pair projection fill (il units).
- Startup (the big win vs the prior version): a memset-fed K=128
  warm-up stream starts the PE/DVFS clock ramp at ~6.7us (right
  after the preamble, no DMA dependency); all tiny tensors ride the
  gpsimd ring so wqk m0 and the x chunks are the FIRST descriptors
  on the sync/scalar rings (the gpsimd ring is served ~7x slower
  under contention -- fine for latency-tolerant 72KB);
  x^T chunks stripe across sync+scalar with pf0 AND pf8 interleaved
  per-chunk (four accumulator chains: 2 psp + 2 borrowed sc slots),
  consuming k-blocks in chunk-arrival order (1,0,3,2); wv splits
  across both rings right behind x; v0 rides inside pair 0's
  attention stream.  First attention exp fires at ~26us vs ~34us.
- The DMA fabric serves ~385GB/s AGGREGATE round-robin across active
  queues; extra queues add nothing, only per-ring FIFO order
  matters.  The full critical prefix (tiny + m0 + m8 + x + m1 + wv =
  4.84MB = ~12.6us) is the startup floor and the prefix PE stream is
  packed against it.
- Pair 7 pads its attention with the first out-projection
  accumulation (k=0..6 of t=0); phase 3 shares the projection PSUM
  pool -- no phase barrier.  Trying to pull MORE of phase 3 into
  pair 5/6 stall slack (opart partials) measured +6us -- the pair
  loop has no genuinely free PE slots.
- Tail: the final out tile is evacuated and DMA'd in quarters on
  alternating engines/rings (sync+scalar only -- a gpsimd SWDGE out
  DMA adds a ~2.2us drain at teardown).
- Run-to-run variance is ~+-2us, dominated by WHERE the DVFS clock
  steps to full rate (16.5-19us from launch; it resists control).
"""

import sys

sys.path.insert(0, "/opt/trn_rl_repo")

import ml_dtypes
import numpy as np

import concourse.bass as bass
import concourse.mybir as mybir
import concourse.tile as tile
from concourse import bacc
from concourse.bass_utils import run_bass_kernel_spmd

B, T, E, H = 8, 1024, 1024, 16
D = E // H  # 64
N_CORES = 8
F32 = mybir.dt.float32
BF16 = mybir.dt.bfloat16
F16 = mybir.dt.float16
# fp16 K=64 row-group concurrent score matmuls (fp32r row-groups worked on
# this HW and ran concurrently; bf16 row-tiling crashed it; fp16 untested)
FP16_SCORES = True
# PE warm-up matmuls before the first projection (DVFS clock ramp)
WARM_N = 12
EXP = mybir.ActivationFunctionType.Exp

_cache = {}


def _build():
    nc = bacc.Bacc("TRN2", target_bir_lowering=False, debug=False,
                   num_devices=N_CORES)

    # ---- DRAM I/O (per core) ----
    xT = nc.dram_tensor("xT", [T + 1, T], F16, kind="ExternalInput").ap()
    w_qkT = nc.dram_tensor("w_qkT", [16, 128, 1024], F16,
                           kind="ExternalInput").ap()
    b_qk = nc.dram_tensor("b_qk", [128, 16], F32, kind="ExternalInput").ap()
    w_vT = nc.dram_tensor("w_vT", [E + 1, E], F16, kind="ExternalInput").ap()
    w_oT = nc.dram_tensor("w_oT", [E, E], F16, kind="ExternalInput").ap()
    tri = nc.dram_tensor("tri", [128, 2 * 128], F16, kind="ExternalInput").ap()
    out = nc.dram_tensor("out", [T, E], F16, kind="ExternalOutput").ap()

    mm = nc.tensor.matmul

    with tile.TileContext(nc) as tc:
        with (
            tc.tile_pool(name="persist", bufs=1) as persist,
        ):
            misc_pool = persist
            # long-lived tensors
            q_sb = persist.tile([128, 8, 1024], F16)      # [e, pair, t]
            if FP16_SCORES:
                # natural stacked k^T [kA; kB]; scores use K=64 row groups
                k_sb = persist.tile([128, 8, 1024], F16)
                kpad = None
            else:
                # per-head zero-padded k^T tiles: [:, p, 0] = [kA; 0],
                # [:, p, 1] = [0; kB] -- K=128 scores, no K-mode switches
                k_sb = None
                kpad = persist.tile([128, 8, 2, 1024], F16)
            # v_ext[:, t, h, :] = [ones(64) | v_h(64)] stationary blocks
            v_ext = persist.tile([128, 8, 16, 128], F16)
            b_qk_sb = misc_pool.tile([128, 16], F32)
            ones_sb = misc_pool.tile([1, 1024], F16)      # ones row
            wsrc = misc_pool.tile([128, 512], F16)        # warm-up source
            tri_sb = misc_pool.tile([128, 2, 128], F16)   # diag mask x2 heads
            brepl = misc_pool.tile([128, 1024], F32)       # v bias replicated

            with (
                tc.tile_pool(name="stat", bufs=1) as stat_pool,
            ):
                xt_pool = wv_pool = wqk_pool = yT_pool = wo_pool = stat_pool
                xt = xt_pool.tile([128, 8, 1024], F16)
                wv = wv_pool.tile([128, 8, 1024], F16)
                wv_bias = wv_pool.tile([1, 1024], F16)
                yT = yT_pool.tile([128, 8, 1024], F16)    # [e, pair, t]
                wo = wo_pool.tile([128, 8, 1024], F16)
                # all qk weight m-tiles; m=0/m=8 land first as small DMAs
                wqk_all = wqk_pool.tile([128, 16, 8, 128], F16)

                # ---- DMA schedule. The fabric serves ~385GB/s AGGREGATE
                # round-robin across active queues (measured); extra queues
                # don't add bandwidth, so only per-ring FIFO order matters.
                # Critical chain: m0 -> x chunks -> m8/m1 -> wv -> m9 ->
                # bulk wqk -> wo.  m0 rides FIRST on sync (the gpsimd ring
                # is served far slower under contention); x stripes across
                # sync+scalar; wv splits across both rings right behind x.
                # warm-up source first on gpsimd (~100ns memset at ~6.5us)
                # so the PE clock ramp starts right after the preamble,
                # not at first-DMA-arrival
                nc.gpsimd.memset(wsrc[:], 1.0)
                if not FP16_SCORES:
                    # zero the pad halves, paced in pair consumption order
                    for pz in range(8):
                        nc.gpsimd.memset(kpad[64:128, pz, 0, :], 0.0)
                        nc.gpsimd.memset(kpad[0:64, pz, 1, :], 0.0)
                nc.sync.dma_start(wv_bias[:], w_vT[E:E + 1, :])
                nc.scalar.dma_start(ones_sb[:], xT[T:T + 1, :])
                nc.gpsimd.dma_start(b_qk_sb[:], b_qk[:])
                nc.gpsimd.dma_start(
                    tri_sb[:].rearrange("p a b -> p (a b)"), tri[:])
                nc.sync.dma_start(
                    wqk_all[:, 0].rearrange("p a b -> p (a b)"), w_qkT[0])

                def xt_chunk(c):
                    return (xt[:, 2 * c:2 * c + 2],
                            xT[256 * c:256 * (c + 1), :].rearrange(
                                "(k p) t -> p k t", p=128))
                # arrival order: chunk1 (scalar) ~12.4us, chunk0 (sync)
                # ~13.8, m8 ~13.7, chunk3/chunk2 ~16.4 -- the interleaved
                # pf0+pf8 consume k-blocks in that order
                nc.sync.dma_start(*xt_chunk(0))
                nc.scalar.dma_start(*xt_chunk(1))
                nc.scalar.dma_start(
                    wqk_all[:, 8].rearrange("p a b -> p (a b)"), w_qkT[8])
                nc.sync.dma_start(*xt_chunk(2))
                nc.scalar.dma_start(*xt_chunk(3))
                nc.scalar.dma_start(
                    wqk_all[:, 1].rearrange("p a b -> p (a b)"), w_qkT[1])
                # wv split across both rings right behind x: full wv by
                # ~23us, ahead of proj_v(0) inside pair 0's stream
                nc.sync.dma_start(
                    wv[:, 0:4],
                    w_vT[0:512, :].rearrange("(k p) e -> p k e", p=128))
                nc.scalar.dma_start(
                    wv[:, 4:8],
                    w_vT[512:1024, :].rearrange("(k p) e -> p k e", p=128))
                # mk1 (=m9) is consumed inside pair 0's attention stream
                nc.scalar.dma_start(
                    wqk_all[:, 9].rearrange("p a b -> p (a b)"), w_qkT[9])
                # bulk wqk on the otherwise-idle sync ring, interleaved in
                # consumption order (mq before mk per pair)
                nc.sync.dma_start(
                    wqk_all[:, 2].rearrange("p a b -> p (a b)"), w_qkT[2])
                nc.sync.dma_start(
                    wqk_all[:, 10].rearrange("p a b -> p (a b)"), w_qkT[10])
                nc.sync.dma_start(
                    wqk_all[:, 3:8].rearrange("p m k c -> p m (k c)"),
                    w_qkT[3:8].rearrange("m p f -> p m f"))
                nc.sync.dma_start(
                    wqk_all[:, 11:16].rearrange("p m k c -> p m (k c)"),
                    w_qkT[11:16].rearrange("m p f -> p m f"))
                nc.scalar.dma_start(
                    wo[:], w_oT[:, :].rearrange("(k p) e -> p k e", p=128))

                with (
                    tc.tile_pool(name="ps_proj", bufs=2, space="PSUM") as psp,
                    tc.tile_pool(name="ps_sc", bufs=2, space="PSUM") as ps_sc,
                    tc.tile_pool(name="ps_ys", bufs=2, space="PSUM") as ps_ys,
                    tc.tile_pool(name="attn", bufs=6) as attn_pool,
                    tc.tile_pool(name="rec", bufs=4) as rec_pool,
                    tc.tile_pool(name="ost", bufs=2) as out_pool,
                ):
                    # ---- v-bias replication: [128, e] = ones^T @ b_v.
                    # pb tiles come from the sc pool (idle until attention)
                    # so the first projections' psp slots are free from the
                    # start; the v_ext memsets queue BEHIND the copies on
                    # the in-order DVE queue (they are not needed until the
                    # first v evac) ----
                    for n in range(2):
                        pb = ps_sc.tile([128, 2, 512], F32, tag="sc",
                                        name=f"pb{n}")
                        mm(pb[:, 0, :], ones_sb[0:1, 0:128],
                           wv_bias[:, 512 * n:512 * (n + 1)])
                        nc.vector.tensor_copy(
                            brepl[:, 512 * n:512 * (n + 1)], pb[:, 0, :])
                    nc.vector.memset(v_ext[:, 0:4, :, 0:64], 1.0)
                    nc.vector.memset(v_ext[:, 4:8, :, 0:64], 1.0)

                    # PE warm-up through the x^T-arrival dead window: keeps
                    # the DVFS clock ramping so the first projections run
                    # at full rate.  Fed from a memset tile (NOT DMA'd
                    # data) so the ramp starts right after the preamble
                    # (~6.7us) instead of at first-DMA-arrival (~9.5us).
                    # Lives in the ys pool (untouched until attention) so
                    # it never waits on the pb evacuations.
                    # full-K warm-ups: the DVFS ramp appears to track real
                    # MAC activity, so K=1 warm-ups don't engage it
                    warm = ps_ys.tile([128, 512], F32, tag="ys",
                                      name="warm")
                    for _ in range(WARM_N):
                        mm(warm[:], wsrc[:, 0:128], wsrc[:, :])

                    def sl_n(n):
                        return slice(512 * n, 512 * (n + 1))

                    def proj_q(m, wsel, n):
                        """One n-half of a q m-tile projection + evac.
                        n=0 evacuates on ACT (Identity+bias), n=1 on DVE:
                        parallel evac chains halve the psp-slot recycle
                        latency that gates the interleaved fill."""
                        ps = psp.tile([128, 512], F32, tag="psp")
                        for k in range(8):
                            mm(ps[:], wsel(k),
                               xt[:, k, 512 * n:512 * (n + 1)],
                               start=(k == 0), stop=(k == 7))
                        dst = q_sb[:, m, 512 * n:512 * (n + 1)]
                        if n == 0:
                            nc.scalar.add(dst, ps[:], b_qk_sb[:, m:m + 1])
                        else:
                            nc.vector.tensor_scalar_add(
                                dst, ps[:], b_qk_sb[:, m:m + 1])

                    def proj_k(p, wsel, n):
                        """One n-half of a k m-tile (m=8+p) + evac."""
                        ps = psp.tile([128, 512], F32, tag="psp")
                        for k in range(8):
                            mm(ps[:], wsel(k),
                               xt[:, k, 512 * n:512 * (n + 1)],
                               start=(k == 0), stop=(k == 7))
                        dst = k_sb[:, p, sl_n(n)]
                        if n == 0:
                            nc.scalar.add(dst, ps[:],
                                          b_qk_sb[:, 8 + p:9 + p])
                        else:
                            nc.vector.tensor_scalar_add(
                                dst, ps[:], b_qk_sb[:, 8 + p:9 + p])

                    def proj_v(t):
                        """v t-tile: psum[t, e] then evac+bias into v_ext."""
                        for n in range(2):
                            ps = psp.tile([128, 512], F32, tag="psp")
                            for k in range(8):
                                mm(ps[:], xt[:, k, 128 * t:128 * (t + 1)],
                                   wv[:, k, 512 * n:512 * (n + 1)],
                                   start=(k == 0), stop=(k == 7))
                            nc.vector.tensor_add(
                                v_ext[:, t, 8 * n:8 * (n + 1), 64:128],
                                ps[:].rearrange("p (a b) -> p a b", a=8),
                                brepl[:, 512 * n:512 * (n + 1)].rearrange(
                                    "p (a b) -> p a b", a=8))

                    # ---- pair-0 projections: pf0 (q) and pf8 (k) fully
                    # interleaved in 2-k blocks, pacing consumption to x^T
                    # chunk arrival order (chunk1, chunk0, chunk3, chunk2).
                    # pf8 accumulates in the sc pool (pb slots free by then)
                    # so four accumulator chains are live at once; v0 rides
                    # inside pair-0's attention stream with v1..v7 ----
                    ps0 = [psp.tile([128, 512], F32, tag="psp",
                                    name=f"pf0_{n}") for n in range(2)]
                    ps8 = [ps_sc.tile([128, 2, 512], F32, tag="sc",
                                      name=f"pf8_{n}") for n in range(2)]
                    kbs = (1, 0, 3, 2)
                    for kb in kbs:
                        for m in (0, 8):
                            ps = ps0 if m == 0 else ps8
                            for n in range(2):
                                dst = ps[n][:] if m == 0 else ps[n][:, 0, :]
                                for k in (2 * kb, 2 * kb + 1):
                                    mm(dst, wqk_all[:, m, k, :],
                                       xt[:, k, 512 * n:512 * (n + 1)],
                                       start=(k == 2 * kbs[0]),
                                       stop=(k == 2 * kbs[-1] + 1))
                    for n in range(2):
                        sl = slice(512 * n, 512 * (n + 1))
                        nc.scalar.add(
                            q_sb[:, 0, sl], ps0[n][:], b_qk_sb[:, 0:1])
                        if FP16_SCORES:
                            nc.vector.tensor_scalar_add(
                                k_sb[:, 0, sl], ps8[n][:, 0, :],
                                b_qk_sb[:, 8:9])
                        else:
                            nc.vector.tensor_scalar_add(
                                kpad[0:64, 0, 0, sl], ps8[n][0:64, 0, :],
                                b_qk_sb[0:64, 8:9])
                            nc.vector.tensor_scalar_add(
                                kpad[64:128, 0, 1, sl],
                                ps8[n][64:128, 0, :],
                                b_qk_sb[64:128, 8:9])
                    for n in range(2):
                        proj_q(1, lambda k: wqk_all[:, 1, k, :], n)

                    # ---- attention, software-pipelined with pair p+1
                    # projections.  Scores are emitted in 2-jt GROUPS of
                    # four b2b fp16 matmuls: each bf16<->fp16 PE mode
                    # switch costs ~100ns, so batching halves the
                    # per-pair transition tax; group g's attn@v drains one
                    # full group later (the sc PSUM pool holds exactly 2
                    # tiles), hiding the ACT exp latency behind real
                    # work. ----
                    def attn_block(p, it, jts, il_pos):
                        """Emit attention for (pair p, query chunk it) over
                        key tiles jts; il_pos maps group index -> list of
                        callables emitted after that group's prev-group
                        attn@v drain (index len(groups) = before the final
                        drain)."""
                        hA, hB = 2 * p, 2 * p + 1
                        psA = ps_ys.tile([128, 512], F32, tag="ys")
                        psB = ps_ys.tile([128, 512], F32, tag="ys")
                        first, last = jts[0], jts[-1]
                        groups = [jts[i:i + 2] for i in range(0, len(jts), 2)]

                        def emit_avs(batch):
                            # NOTE: K=64 row-group pairing of attn@v (to
                            # stay in K=64 mode through the attention
                            # phase) crashes the HW: consecutive slots
                            # write the same PSUM bank from both array
                            # halves.  K=128 it stays.
                            for jt, lo, at in batch:
                                st = (jt == first)
                                sp = (jt == last)
                                mm(psA[:, lo:512], v_ext[:, jt, hA, :],
                                   at[:, 0, lo:512], start=st, stop=sp)
                                mm(psB[:, lo:512], v_ext[:, jt, hB, :],
                                   at[:, 1, lo:512], start=st, stop=sp)

                        prev = []
                        for g, grp in enumerate(groups):
                            cur = []
                            for jt in grp:
                                r = jt - 4 * it
                                lo = 128 * r if r > 0 else 0
                                sc = ps_sc.tile([128, 2, 512], F32,
                                                tag="sc")
                                at = attn_pool.tile([128, 2, 512], F16)
                                if FP16_SCORES:
                                    # fp16 K=64 matmuls on PE row groups
                                    # 0/64: the two heads execute
                                    # CONCURRENTLY
                                    mm(sc[:, 0, lo:512],
                                       k_sb[0:64, p,
                                            128 * jt:128 * (jt + 1)],
                                       q_sb[0:64, p,
                                            512 * it + lo:512 * (it + 1)])
                                    mm(sc[:, 1, lo:512],
                                       k_sb[64:128, p,
                                            128 * jt:128 * (jt + 1)],
                                       q_sb[64:128, p,
                                            512 * it + lo:512 * (it + 1)])
                                else:
                                    # zero-padded K=128, serial per head
                                    mm(sc[:, 0, lo:512],
                                       kpad[:, p, 0,
                                            128 * jt:128 * (jt + 1)],
                                       q_sb[:, p,
                                            512 * it + lo:512 * (it + 1)])
                                    mm(sc[:, 1, lo:512],
                                       kpad[:, p, 1,
                                            128 * jt:128 * (jt + 1)],
                                       q_sb[:, p,
                                            512 * it + lo:512 * (it + 1)])
                                cur.append((jt, lo, sc, at))
                            for jt, lo, sc, at in cur:
                                # joint exp over both heads, one per jt
                                nc.scalar.activation(at[:, :, lo:512],
                                                     sc[:, :, lo:512], EXP,
                                                     scale=0.125)
                                r = jt - 4 * it
                                if 0 <= r <= 3:
                                    # tri mask stays on DVE (gpsimd's
                                    # latency lands on the critical path)
                                    nc.vector.tensor_mul(
                                        at[:, :, lo:lo + 128],
                                        at[:, :, lo:lo + 128], tri_sb[:])
                            emit_avs([(jt, lo, at)
                                      for jt, lo, sc, at in prev])
                            for fn in il_pos.get(g, ()):
                                fn()
                            prev = cur
                        for fn in il_pos.get(len(groups), ()):
                            fn()
                        emit_avs([(jt, lo, at)
                                  for jt, lo, sc, at in prev])

                        recA = rec_pool.tile([64, 512], F32, tag="rec")
                        recB = rec_pool.tile([64, 512], F32, tag="rec")
                        nc.vector.reciprocal_approx_fast(recA[:], psA[0:64, :])
                        nc.vector.reciprocal_approx_fast(recB[:], psB[0:64, :])
                        sl = slice(512 * it, 512 * (it + 1))
                        nc.vector.tensor_mul(
                            yT[0:64, p, sl], psA[64:128, :], recA[:])
                        nc.vector.tensor_mul(
                            yT[64:128, p, sl], psB[64:128, :], recB[:])

                    p3_partial = {}

                    def p3_acc(n, ks):
                        # open/extend the t=0 out-projection accumulation
                        # (k=0..6 only: pair-7 yT is not final yet);
                        # finished in phase 3
                        if n not in p3_partial:
                            p3_partial[n] = psp.tile([128, 512], F32,
                                                     tag="psp",
                                                     name=f"p3p{n}")
                        ps = p3_partial[n]
                        for k in ks:
                            mm(ps[:], yT[:, k, 0:128],
                               wo[:, k, 512 * n:512 * (n + 1)],
                               start=(k == 0), stop=False)

                    def pq(m, n):
                        return lambda: proj_q(
                            m, lambda k: wqk_all[:, m, k, :], n)

                    def pk(pp, n):
                        return lambda: proj_k(
                            pp, lambda k: wqk_all[:, 8 + pp, k, :], n)

                    for p in range(8):
                        nxt = p + 1
                        if p == 0:
                            il0 = {0: [lambda: proj_v(0),
                                       lambda: proj_v(1)],
                                   1: [lambda: proj_v(2)],
                                   2: [lambda: proj_v(3)]}
                            il1 = {0: [lambda: proj_v(4)],
                                   1: [lambda: proj_v(5)],
                                   2: [lambda: proj_v(6), pk(1, 0)],
                                   3: [lambda: proj_v(7), pk(1, 1)]}
                        elif nxt < 8:
                            il0 = {0: [pq(nxt, 0)], 2: [pq(nxt, 1)]}
                            il1 = {0: [pk(nxt, 0)], 4: [pk(nxt, 1)]}
                        else:
                            il0 = {0: [lambda: p3_acc(0, range(4))],
                                   2: [lambda: p3_acc(0, range(4, 7))]}
                            il1 = {0: [lambda: p3_acc(1, range(4))],
                                   4: [lambda: p3_acc(1, range(4, 7))]}
                        attn_block(p, 0, range(4), il0)
                        attn_block(p, 1, range(8), il1)

                    # ------------ Phase 3: out projection ----------------
                    for t in range(8):
                        st = out_pool.tile([128, 2, 512], F16)
                        for n in range(2):
                            if t == 0:
                                ps = p3_partial.pop(n)
                                mm(ps[:], yT[:, 7, 0:128],
                                   wo[:, 7, 512 * n:512 * (n + 1)],
                                   start=False, stop=True)
                            else:
                                ps = psp.tile([128, 512], F32, tag="psp")
                                for k in range(8):
                                    mm(ps[:],
                                       yT[:, k, 128 * t:128 * (t + 1)],
                                       wo[:, k, 512 * n:512 * (n + 1)],
                                       start=(k == 0), stop=(k == 7))
                            if t == 7:
                                # final tile: quarter-granularity evac+DMA
                                # on parallel engines/rings to shrink the
                                # kernel tail
                                for h in range(2):
                                    sl = slice(256 * h, 256 * (h + 1))
                                    if h == 0:
                                        nc.scalar.copy(st[:, n, sl],
                                                       ps[:, sl])
                                    else:
                                        nc.vector.tensor_copy(st[:, n, sl],
                                                              ps[:, sl])
                                    ring = nc.sync if h == 0 else nc.scalar
                                    ring.dma_start(
                                        out[128 * t:128 * (t + 1),
                                            512 * n + 256 * h:
                                            512 * n + 256 * (h + 1)],
                                        st[:, n, sl])
                            elif n == 0:
                                nc.scalar.copy(st[:, 0, :], ps[:])
                            else:
                                nc.vector.tensor_copy(st[:, 1, :], ps[:])
                        if t < 7:
                            nc.sync.dma_start(
                                out[128 * t:128 * (t + 1), :],
                                st[:].rearrange("p a b -> p (a b)"))

    nc.compile()
    return nc


def _host_prep(x, w_qkv, b_qkv, w_out):
    bf = np.float16
    x = np.asarray(x, dtype=np.float32)
    w_qkv = np.asarray(w_qkv, dtype=np.float32)
    b_qkv = np.asarray(b_qkv, dtype=np.float32)
    w_out = np.asarray(w_out, dtype=np.float32)

    # [m, p, k, c] pre-tiled so each m-tile is one contiguous DMA
    w_qkT = np.ascontiguousarray(
        w_qkv[:2 * E].T.reshape(8, 128, 16, 128).transpose(2, 1, 0, 3)
    ).reshape(16, 128, 1024).astype(bf)
    b_qk = np.ascontiguousarray(
        b_qkv[:2 * E].reshape(16, 128).T).astype(np.float32)     # [128, 16]
    w_vT = np.concatenate(
        [w_qkv[2 * E:].T, b_qkv[2 * E:][None, :]], axis=0).astype(bf)
    w_oT = np.ascontiguousarray(w_out.T).astype(bf)              # [E, E]

    j = np.arange(128)[:, None]
    i = np.arange(128)[None, :]
    tri1 = (j <= i).astype(np.float32)
    tri = np.concatenate([tri1, tri1], axis=1).astype(bf)        # [128, 256]

    ones = np.ones((1, T), dtype=np.float32)
    per_core = []
    for c in range(N_CORES):
        xTc = np.concatenate([x[c].T, ones], axis=0).astype(bf)
        per_core.append({
            "xT": xTc, "w_qkT": w_qkT, "b_qk": b_qk, "w_vT": w_vT,
            "w_oT": w_oT, "tri": tri,
        })
    return per_core


def kernel(x, w_qkv, b_qkv, w_out, b_out, cos_tab, sin_tab):
    # cos_tab/sin_tab unused: the module applies the identical rotation R to
    # q and k at every position and R R^T = I cancels inside q @ k^T.
    if "nc" not in _cache:
        _cache["nc"] = _build()
    nc = _cache["nc"]
    in_maps = _host_prep(x, w_qkv, b_qkv, w_out)
    res = run_bass_kernel_spmd(nc, in_maps, list(range(N_CORES)),
                               trace=False)
    out = np.stack([res.results[c]["out"] for c in range(N_CORES)], axis=0)
    return (out + np.asarray(b_out, dtype=np.float32)).astype(np.float32)


def run_traced(x, w_qkv, b_qkv, w_out, b_out, cos_tab, sin_tab):
    """Like kernel() but with NTFF profiling; returns (out, exec_time_ns,
    trace_path)."""
    if "nc" not in _cache:
        _cache["nc"] = _build()
    nc = _cache["nc"]
    in_maps = _host_prep(x, w_qkv, b_qkv, w_out)
    res = run_bass_kernel_spmd(nc, in_maps, list(range(N_CORES)), trace=True)
    out = np.stack([res.results[c]["out"] for c in range(N_CORES)], axis=0)
    out = (out + np.asarray(b_out, dtype=np.float32)).astype(np.float32)
    trace_path = None
    if res.instructions_and_trace is not None:
        trace_path = res.instructions_and_trace[1]
    return out, res.exec_time_ns, trace_path



# revision 41
# speedup vs baseline: 1.0088x; 1.0088x over previous
"""Trainium2 Bass kernel for nn_AttentionBlock_15470472200943.

Causal multi-head attention block (B=8, T=1024, E=1024, H=16, D=64),
data-parallel: one batch element per NeuronCore across 8 cores.
~195us HW exec (profiled), down from the 205us prior best / 277us
original baseline.

Key design points:
- RoPE skipped: the module applies the identical rotation R to q and k
  at every position and R R^T = I cancels inside q @ k^T.
- Everything on the PE is fp16 (NOT bf16): same 1 col/cycle rate,
  3 more mantissa bits (rel_l2 5.5e-4 vs 3.9e-3), and it matches the
  fp16 score matmuls.  Output is stored fp16 and upconverted on host.
- Scores: fp16 K=64 matmuls on PE row groups 0/64 -- the two heads of
  a pair launch ~3ns apart and execute CONCURRENTLY on the array
  halves, halving score cost.  Each K=128<->K=64 mode edge exposes a
  ~100ns LDWEIGHTS (~1.9us/pair); attempts to remove it all fail:
  K=64-paired attn@v crashes the HW (two row-group streams writing
  one PSUM bank), zero-padded K=128 scores cost more than the tax,
  and emission-order grouping is ignored (the Tile scheduler is a
  ready-list scheduler, not in-order).
- Scores/exp/attn@v restricted to causally-live columns at 128-col
  granularity; only diagonal tiles get an elementwise tri-mask (DVE).
- No bias-via-matmul: qk bias folds into the DVE PSUM-evacuation,
  v bias is a 128-replicated SBUF tile added during the v evac, and
  the out-projection bias is added on the host.
- Softmax denominator comes out of the attn@v matmul itself
  (stationary [ones(64) | v_h(64)]); no max-subtraction (scores
  bounded, exp safe); 1/sqrt(D) folds into the exp scale.
- Engine placement: PE matmuls only; ACT does exp ONLY during
  attention (one joint exp per key-tile); DVE does PSUM evacs + tri
  masks + reciprocal + normalize.  The three are balanced within
  ~15% of each other in the pair loop -- moving work between them
  (tri->gpsimd, exp-split, sc->SBUF copies) makes some engine the
  new bottleneck.
- Attention emits scores in 2-jt groups with attn@v drained one full
  group later (sc PSUM pool = exactly 2 tiles), hiding exp latency
  behind next-pair projection fill (il units).
- Startup (the big win vs the prior version): a memset-fed K=128
  warm-up stream starts the PE/DVFS clock ramp at ~6.7us (right
  after the preamble, no DMA dependency); all tiny tensors ride the
  gpsimd ring so wqk m0 and the x chunks are the FIRST descriptors
  on the sync/scalar rings (the gpsimd ring is served ~7x slower
  under contention -- fine for latency-tolerant 72KB);
  x^T chunks stripe across sync+scalar with pf0 AND pf8 interleaved
  per-chunk (four accumulator chains: 2 psp + 2 borrowed sc slots),
  consuming k-blocks in chunk-arrival order (1,0,3,2); wv splits
  across both rings right behind x; v0 rides inside pair 0's
  attention stream.  First attention exp fires at ~26us vs ~34us.
- The DMA fabric serves ~385GB/s AGGREGATE round-robin across active
  queues; extra queues add nothing, only per-ring FIFO order
  matters.  The full critical prefix (tiny + m0 + m8 + x + m1 + wv =
  4.84MB = ~12.6us) is the startup floor and the prefix PE stream is
  packed against it.
- Pair 7 pads its attention with the first out-projection
  accumulation (k=0..6 of t=0); phase 3 shares the projection PSUM
  pool -- no phase barrier.  Trying to pull MORE of phase 3 into
  pair 5/6 stall slack (opart partials) measured +6us -- the pair
  loop has no genuinely free PE slots.
- Tail: the final out tile is evacuated and DMA'd in quarters on
  alternating engines/rings (sync+scalar only -- a gpsimd SWDGE out
  DMA adds a ~2.2us drain at teardown).
- Run-to-run variance is ~+-2us, dominated by WHERE the DVFS clock
  steps to full rate (16.5-19us from launch; it resists control).
"""

import sys

sys.path.insert(0, "/opt/trn_rl_repo")

import ml_dtypes
import numpy as np

import concourse.bass as bass
import concourse.mybir as mybir
import concourse.tile as tile
from concourse import bacc
from concourse.bass_utils import run_bass_kernel_spmd

B, T, E, H = 8, 1024, 1024, 16
D = E // H  # 64
N_CORES = 8
F32 = mybir.dt.float32
BF16 = mybir.dt.bfloat16
F16 = mybir.dt.float16
# fp16 K=64 row-group concurrent score matmuls (fp32r row-groups worked on
# this HW and ran concurrently; bf16 row-tiling crashed it; fp16 untested)
FP16_SCORES = True
# PE warm-up matmuls before the first projection (DVFS clock ramp)
WARM_N = 12
EXP = mybir.ActivationFunctionType.Exp

_cache = {}


def _build():
    nc = bacc.Bacc("TRN2", target_bir_lowering=False, debug=False,
                   num_devices=N_CORES)

    # ---- DRAM I/O (per core) ----
    xT = nc.dram_tensor("xT", [T + 1, T], F16, kind="ExternalInput").ap()
    w_qkT = nc.dram_tensor("w_qkT", [16, 128, 1024], F16,
                           kind="ExternalInput").ap()
    b_qk = nc.dram_tensor("b_qk", [128, 16], F32, kind="ExternalInput").ap()
    w_vT = nc.dram_tensor("w_vT", [E + 1, E], F16, kind="ExternalInput").ap()
    w_oT = nc.dram_tensor("w_oT", [E, E], F16, kind="ExternalInput").ap()
    tri = nc.dram_tensor("tri", [128, 2 * 128], F16, kind="ExternalInput").ap()
    out = nc.dram_tensor("out", [T, E], F16, kind="ExternalOutput").ap()

    mm = nc.tensor.matmul

    with tile.TileContext(nc) as tc:
        with (
            tc.tile_pool(name="persist", bufs=1) as persist,
        ):
            misc_pool = persist
            # long-lived tensors
            q_sb = persist.tile([128, 8, 1024], F16)      # [e, pair, t]
            if FP16_SCORES:
                # natural stacked k^T [kA; kB]; scores use K=64 row groups
                k_sb = persist.tile([128, 8, 1024], F16)
                kpad = None
            else:
                # per-head zero-padded k^T tiles: [:, p, 0] = [kA; 0],
                # [:, p, 1] = [0; kB] -- K=128 scores, no K-mode switches
                k_sb = None
                kpad = persist.tile([128, 8, 2, 1024], F16)
            # v_ext[:, t, h, :] = [ones(64) | v_h(64)] stationary blocks
            v_ext = persist.tile([128, 8, 16, 128], F16)
            b_qk_sb = misc_pool.tile([128, 16], F32)
            ones_sb = misc_pool.tile([1, 1024], F16)      # ones row
            wsrc = misc_pool.tile([128, 512], F16)        # warm-up source
            tri_sb = misc_pool.tile([128, 2, 128], F16)   # diag mask x2 heads
            brepl = misc_pool.tile([128, 1024], F32)       # v bias replicated

            with (
                tc.tile_pool(name="stat", bufs=1) as stat_pool,
            ):
                xt_pool = wv_pool = wqk_pool = yT_pool = wo_pool = stat_pool
                xt = xt_pool.tile([128, 8, 1024], F16)
                wv = wv_pool.tile([128, 8, 1024], F16)
                wv_bias = wv_pool.tile([1, 1024], F16)
                yT = yT_pool.tile([128, 8, 1024], F16)    # [e, pair, t]
                wo = wo_pool.tile([128, 8, 1024], F16)
                # all qk weight m-tiles; m=0/m=8 land first as small DMAs
                wqk_all = wqk_pool.tile([128, 16, 8, 128], F16)

                # ---- DMA schedule. The fabric serves ~385GB/s AGGREGATE
                # round-robin across active queues (measured); extra queues
                # don't add bandwidth, so only per-ring FIFO order matters.
                # Critical chain: m0 -> x chunks -> m8/m1 -> wv -> m9 ->
                # bulk wqk -> wo.  m0 rides FIRST on sync (the gpsimd ring
                # is served far slower under contention); x stripes across
                # sync+scalar; wv splits across both rings right behind x.
                # warm-up source first on gpsimd (~100ns memset at ~6.5us)
                # so the PE clock ramp starts right after the preamble,
                # not at first-DMA-arrival
                nc.gpsimd.memset(wsrc[:], 1.0)
                if not FP16_SCORES:
                    # zero the pad halves, paced in pair consumption order
                    for pz in range(8):
                        nc.gpsimd.memset(kpad[64:128, pz, 0, :], 0.0)
                        nc.gpsimd.memset(kpad[0:64, pz, 1, :], 0.0)
                nc.sync.dma_start(wv_bias[:], w_vT[E:E + 1, :])
                nc.scalar.dma_start(ones_sb[:], xT[T:T + 1, :])
                nc.gpsimd.dma_start(b_qk_sb[:], b_qk[:])
                nc.gpsimd.dma_start(
                    tri_sb[:].rearrange("p a b -> p (a b)"), tri[:])
                nc.sync.dma_start(
                    wqk_all[:, 0].rearrange("p a b -> p (a b)"), w_qkT[0])

                def xt_chunk(c):
                    return (xt[:, 2 * c:2 * c + 2],
                            xT[256 * c:256 * (c + 1), :].rearrange(
                                "(k p) t -> p k t", p=128))
                # arrival order: chunk1 (scalar) ~12.4us, chunk0 (sync)
                # ~13.8, m8 ~13.7, chunk3/chunk2 ~16.4 -- the interleaved
                # pf0+pf8 consume k-blocks in that order
                nc.sync.dma_start(*xt_chunk(0))
                nc.scalar.dma_start(*xt_chunk(1))
                nc.scalar.dma_start(
                    wqk_all[:, 8].rearrange("p a b -> p (a b)"), w_qkT[8])
                nc.sync.dma_start(*xt_chunk(2))
                nc.scalar.dma_start(*xt_chunk(3))
                nc.scalar.dma_start(
                    wqk_all[:, 1].rearrange("p a b -> p (a b)"), w_qkT[1])
                # wv split across both rings right behind x: full wv by
                # ~23us, ahead of proj_v(0) inside pair 0's stream
                nc.sync.dma_start(
                    wv[:, 0:4],
                    w_vT[0:512, :].rearrange("(k p) e -> p k e", p=128))
                nc.scalar.dma_start(
                    wv[:, 4:8],
                    w_vT[512:1024, :].rearrange("(k p) e -> p k e", p=128))
                # mk1 (=m9) is consumed inside pair 0's attention stream
                nc.scalar.dma_start(
                    wqk_all[:, 9].rearrange("p a b -> p (a b)"), w_qkT[9])
                # bulk wqk on the otherwise-idle sync ring, interleaved in
                # consumption order (mq before mk per pair)
                nc.sync.dma_start(
                    wqk_all[:, 2].rearrange("p a b -> p (a b)"), w_qkT[2])
                nc.sync.dma_start(
                    wqk_all[:, 10].rearrange("p a b -> p (a b)"), w_qkT[10])
                nc.sync.dma_start(
                    wqk_all[:, 3:8].rearrange("p m k c -> p m (k c)"),
                    w_qkT[3:8].rearrange("m p f -> p m f"))
                nc.sync.dma_start(
                    wqk_all[:, 11:16].rearrange("p m k c -> p m (k c)"),
                    w_qkT[11:16].rearrange("m p f -> p m f"))
                nc.scalar.dma_start(
                    wo[:], w_oT[:, :].rearrange("(k p) e -> p k e", p=128))

                with (
                    tc.tile_pool(name="ps_proj", bufs=2, space="PSUM") as psp,
                    tc.tile_pool(name="ps_sc", bufs=2, space="PSUM") as ps_sc,
                    tc.tile_pool(name="ps_ys", bufs=2, space="PSUM") as ps_ys,
                    tc.tile_pool(name="attn", bufs=6) as attn_pool,
                    tc.tile_pool(name="rec", bufs=4) as rec_pool,
                    tc.tile_pool(name="ost", bufs=2) as out_pool,
                ):
                    # ---- v-bias replication: [128, e] = ones^T @ b_v.
                    # pb tiles come from the sc pool (idle until attention)
                    # so the first projections' psp slots are free from the
                    # start; the v_ext memsets queue BEHIND the copies on
                    # the in-order DVE queue (they are not needed until the
                    # first v evac) ----
                    for n in range(2):
                        pb = ps_sc.tile([128, 2, 512], F32, tag="sc",
                                        name=f"pb{n}")
                        mm(pb[:, 0, :], ones_sb[0:1, 0:128],
                           wv_bias[:, 512 * n:512 * (n + 1)])
                        nc.vector.tensor_copy(
                            brepl[:, 512 * n:512 * (n + 1)], pb[:, 0, :])
                    nc.vector.memset(v_ext[:, 0:4, :, 0:64], 1.0)
                    nc.vector.memset(v_ext[:, 4:8, :, 0:64], 1.0)

                    # PE warm-up through the x^T-arrival dead window: keeps
                    # the DVFS clock ramping so the first projections run
                    # at full rate.  Fed from a memset tile (NOT DMA'd
                    # data) so the ramp starts right after the preamble
                    # (~6.7us) instead of at first-DMA-arrival (~9.5us).
                    # Lives in the ys pool (untouched until attention) so
                    # it never waits on the pb evacuations.
                    # full-K warm-ups: the DVFS ramp appears to track real
                    # MAC activity, so K=1 warm-ups don't engage it
                    warm = ps_ys.tile([128, 512], F32, tag="ys",
                                      name="warm")
                    for _ in range(WARM_N):
                        mm(warm[:], wsrc[:, 0:128], wsrc[:, :])

                    def sl_n(n):
                        return slice(512 * n, 512 * (n + 1))

                    def proj_q(m, wsel, n):
                        """One n-half of a q m-tile projection + evac."""
                        ps = psp.tile([128, 512], F32, tag="psp")
                        for k in range(8):
                            mm(ps[:], wsel(k),
                               xt[:, k, 512 * n:512 * (n + 1)],
                               start=(k == 0), stop=(k == 7))
                        nc.vector.tensor_scalar_add(
                            q_sb[:, m, 512 * n:512 * (n + 1)], ps[:],
                            b_qk_sb[:, m:m + 1])

                    def proj_k(p, wsel, n):
                        """One n-half of a k m-tile (m=8+p) + evac."""
                        ps = psp.tile([128, 512], F32, tag="psp")
                        for k in range(8):
                            mm(ps[:], wsel(k),
                               xt[:, k, 512 * n:512 * (n + 1)],
                               start=(k == 0), stop=(k == 7))
                        nc.vector.tensor_scalar_add(
                            k_sb[:, p, sl_n(n)], ps[:],
                            b_qk_sb[:, 8 + p:9 + p])

                    def proj_v(t):
                        """v t-tile: psum[t, e] then evac+bias into v_ext."""
                        for n in range(2):
                            ps = psp.tile([128, 512], F32, tag="psp")
                            for k in range(8):
                                mm(ps[:], xt[:, k, 128 * t:128 * (t + 1)],
                                   wv[:, k, 512 * n:512 * (n + 1)],
                                   start=(k == 0), stop=(k == 7))
                            nc.vector.tensor_add(
                                v_ext[:, t, 8 * n:8 * (n + 1), 64:128],
                                ps[:].rearrange("p (a b) -> p a b", a=8),
                                brepl[:, 512 * n:512 * (n + 1)].rearrange(
                                    "p (a b) -> p a b", a=8))

                    # ---- pair-0 projections: pf0 (q) and pf8 (k) fully
                    # interleaved in 2-k blocks, pacing consumption to x^T
                    # chunk arrival order (chunk1, chunk0, chunk3, chunk2).
                    # pf8 accumulates in the sc pool (pb slots free by then)
                    # so four accumulator chains are live at once; v0 rides
                    # inside pair-0's attention stream with v1..v7 ----
                    ps0 = [psp.tile([128, 512], F32, tag="psp",
                                    name=f"pf0_{n}") for n in range(2)]
                    ps8 = [ps_sc.tile([128, 2, 512], F32, tag="sc",
                                      name=f"pf8_{n}") for n in range(2)]
                    kbs = (1, 0, 3, 2)
                    for kb in kbs:
                        for m in (0, 8):
                            ps = ps0 if m == 0 else ps8
                            for n in range(2):
                                dst = ps[n][:] if m == 0 else ps[n][:, 0, :]
                                for k in (2 * kb, 2 * kb + 1):
                                    mm(dst, wqk_all[:, m, k, :],
                                       xt[:, k, 512 * n:512 * (n + 1)],
                                       start=(k == 2 * kbs[0]),
                                       stop=(k == 2 * kbs[-1] + 1))
                    for n in range(2):
                        sl = slice(512 * n, 512 * (n + 1))
                        nc.vector.tensor_scalar_add(
                            q_sb[:, 0, sl], ps0[n][:], b_qk_sb[:, 0:1])
                        if FP16_SCORES:
                            nc.vector.tensor_scalar_add(
                                k_sb[:, 0, sl], ps8[n][:, 0, :],
                                b_qk_sb[:, 8:9])
                        else:
                            nc.vector.tensor_scalar_add(
                                kpad[0:64, 0, 0, sl], ps8[n][0:64, 0, :],
                                b_qk_sb[0:64, 8:9])
                            nc.vector.tensor_scalar_add(
                                kpad[64:128, 0, 1, sl],
                                ps8[n][64:128, 0, :],
                                b_qk_sb[64:128, 8:9])
                    for n in range(2):
                        proj_q(1, lambda k: wqk_all[:, 1, k, :], n)

                    # ---- attention, software-pipelined with pair p+1
                    # projections.  Scores are emitted in 2-jt GROUPS of
                    # four b2b fp16 matmuls: each bf16<->fp16 PE mode
                    # switch costs ~100ns, so batching halves the
                    # per-pair transition tax; group g's attn@v drains one
                    # full group later (the sc PSUM pool holds exactly 2
                    # tiles), hiding the ACT exp latency behind real
                    # work. ----
                    def attn_block(p, it, jts, il_pos):
                        """Emit attention for (pair p, query chunk it) over
                        key tiles jts; il_pos maps group index -> list of
                        callables emitted after that group's prev-group
                        attn@v drain (index len(groups) = before the final
                        drain)."""
                        hA, hB = 2 * p, 2 * p + 1
                        psA = ps_ys.tile([128, 512], F32, tag="ys")
                        psB = ps_ys.tile([128, 512], F32, tag="ys")
                        first, last = jts[0], jts[-1]
                        groups = [jts[i:i + 2] for i in range(0, len(jts), 2)]

                        def emit_avs(batch):
                            # NOTE: K=64 row-group pairing of attn@v (to
                            # stay in K=64 mode through the attention
                            # phase) crashes the HW: consecutive slots
                            # write the same PSUM bank from both array
                            # halves.  K=128 it stays.
                            for jt, lo, at in batch:
                                st = (jt == first)
                                sp = (jt == last)
                                mm(psA[:, lo:512], v_ext[:, jt, hA, :],
                                   at[:, 0, lo:512], start=st, stop=sp)
                                mm(psB[:, lo:512], v_ext[:, jt, hB, :],
                                   at[:, 1, lo:512], start=st, stop=sp)

                        prev = []
                        for g, grp in enumerate(groups):
                            cur = []
                            for jt in grp:
                                r = jt - 4 * it
                                lo = 128 * r if r > 0 else 0
                                sc = ps_sc.tile([128, 2, 512], F32,
                                                tag="sc")
                                at = attn_pool.tile([128, 2, 512], F16)
                                if FP16_SCORES:
                                    # fp16 K=64 matmuls on PE row groups
                                    # 0/64: the two heads execute
                                    # CONCURRENTLY
                                    mm(sc[:, 0, lo:512],
                                       k_sb[0:64, p,
                                            128 * jt:128 * (jt + 1)],
                                       q_sb[0:64, p,
                                            512 * it + lo:512 * (it + 1)])
                                    mm(sc[:, 1, lo:512],
                                       k_sb[64:128, p,
                                            128 * jt:128 * (jt + 1)],
                                       q_sb[64:128, p,
                                            512 * it + lo:512 * (it + 1)])
                                else:
                                    # zero-padded K=128, serial per head
                                    mm(sc[:, 0, lo:512],
                                       kpad[:, p, 0,
                                            128 * jt:128 * (jt + 1)],
                                       q_sb[:, p,
                                            512 * it + lo:512 * (it + 1)])
                                    mm(sc[:, 1, lo:512],
                                       kpad[:, p, 1,
                                            128 * jt:128 * (jt + 1)],
                                       q_sb[:, p,
                                            512 * it + lo:512 * (it + 1)])
                                cur.append((jt, lo, sc, at))
                            for jt, lo, sc, at in cur:
                                # joint exp over both heads, one per jt
                                nc.scalar.activation(at[:, :, lo:512],
                                                     sc[:, :, lo:512], EXP,
                                                     scale=0.125)
                                r = jt - 4 * it
                                if 0 <= r <= 3:
                                    # tri mask stays on DVE (gpsimd's
                                    # latency lands on the critical path)
                                    nc.vector.tensor_mul(
                                        at[:, :, lo:lo + 128],
                                        at[:, :, lo:lo + 128], tri_sb[:])
                            emit_avs([(jt, lo, at)
                                      for jt, lo, sc, at in prev])
                            for fn in il_pos.get(g, ()):
                                fn()
                            prev = cur
                        for fn in il_pos.get(len(groups), ()):
                            fn()
                        emit_avs([(jt, lo, at)
                                  for jt, lo, sc, at in prev])

                        recA = rec_pool.tile([64, 512], F32, tag="rec")
                        recB = rec_pool.tile([64, 512], F32, tag="rec")
                        nc.vector.reciprocal_approx_fast(recA[:], psA[0:64, :])
                        nc.vector.reciprocal_approx_fast(recB[:], psB[0:64, :])
                        sl = slice(512 * it, 512 * (it + 1))
                        nc.vector.tensor_mul(
                            yT[0:64, p, sl], psA[64:128, :], recA[:])
                        nc.vector.tensor_mul(
                            yT[64:128, p, sl], psB[64:128, :], recB[:])

                    p3_partial = {}

                    def p3_acc(n, ks):
                        # open/extend the t=0 out-projection accumulation
                        # (k=0..6 only: pair-7 yT is not final yet);
                        # finished in phase 3
                        if n not in p3_partial:
                            p3_partial[n] = psp.tile([128, 512], F32,
                                                     tag="psp",
                                                     name=f"p3p{n}")
                        ps = p3_partial[n]
                        for k in ks:
                            mm(ps[:], yT[:, k, 0:128],
                               wo[:, k, 512 * n:512 * (n + 1)],
                               start=(k == 0), stop=False)

                    def pq(m, n):
                        return lambda: proj_q(
                            m, lambda k: wqk_all[:, m, k, :], n)

                    def pk(pp, n):
                        return lambda: proj_k(
                            pp, lambda k: wqk_all[:, 8 + pp, k, :], n)

                    for p in range(8):
                        nxt = p + 1
                        if p == 0:
                            il0 = {0: [lambda: proj_v(0),
                                       lambda: proj_v(1)],
                                   1: [lambda: proj_v(2)],
                                   2: [lambda: proj_v(3)]}
                            il1 = {0: [lambda: proj_v(4)],
                                   1: [lambda: proj_v(5)],
                                   2: [lambda: proj_v(6), pk(1, 0)],
                                   3: [lambda: proj_v(7), pk(1, 1)]}
                        elif nxt < 8:
                            il0 = {0: [pq(nxt, 0)], 2: [pq(nxt, 1)]}
                            il1 = {0: [pk(nxt, 0)], 4: [pk(nxt, 1)]}
                        else:
                            il0 = {0: [lambda: p3_acc(0, range(4))],
                                   2: [lambda: p3_acc(0, range(4, 7))]}
                            il1 = {0: [lambda: p3_acc(1, range(4))],
                                   4: [lambda: p3_acc(1, range(4, 7))]}
                        attn_block(p, 0, range(4), il0)
                        attn_block(p, 1, range(8), il1)

                    # ------------ Phase 3: out projection ----------------
                    for t in range(8):
                        st = out_pool.tile([128, 2, 512], F16)
                        for n in range(2):
                            if t == 0:
                                ps = p3_partial.pop(n)
                                mm(ps[:], yT[:, 7, 0:128],
                                   wo[:, 7, 512 * n:512 * (n + 1)],
                                   start=False, stop=True)
                            else:
                                ps = psp.tile([128, 512], F32, tag="psp")
                                for k in range(8):
                                    mm(ps[:],
                                       yT[:, k, 128 * t:128 * (t + 1)],
                                       wo[:, k, 512 * n:512 * (n + 1)],
                                       start=(k == 0), stop=(k == 7))
                            if t == 7:
                                # final tile: quarter-granularity evac+DMA
                                # on parallel engines/rings to shrink the
                                # kernel tail
                                for h in range(2):
                                    sl = slice(256 * h, 256 * (h + 1))
                                    if h == 0:
                                        nc.scalar.copy(st[:, n, sl],
                                                       ps[:, sl])
                                    else:
                                        nc.vector.tensor_copy(st[:, n, sl],
                                                              ps[:, sl])
                                    ring = nc.sync if h == 0 else nc.scalar
                                    ring.dma_start(
                                        out[128 * t:128 * (t + 1),
                                            512 * n + 256 * h:
                                            512 * n + 256 * (h + 1)],
                                        st[:, n, sl])
                            elif n == 0:
                                nc.scalar.copy(st[:, 0, :], ps[:])
                            else:
                                nc.vector.tensor_copy(st[:, 1, :], ps[:])
                        if t < 7:
                            nc.sync.dma_start(
                                out[128 * t:128 * (t + 1), :],
                                st[:].rearrange("p a b -> p (a b)"))

    nc.compile()
    return nc


def _host_prep(x, w_qkv, b_qkv, w_out):
    bf = np.float16
    x = np.asarray(x, dtype=np.float32)
    w_qkv = np.asarray(w_qkv, dtype=np.float32)
    b_qkv = np.asarray(b_qkv, dtype=np.float32)
    w_out = np.asarray(w_out, dtype=np.float32)

    # [m, p, k, c] pre-tiled so each m-tile is one contiguous DMA
    w_qkT = np.ascontiguousarray(
        w_qkv[:2 * E].T.reshape(8, 128, 16, 128).transpose(2, 1, 0, 3)
    ).reshape(16, 128, 1024).astype(bf)
    b_qk = np.ascontiguousarray(
        b_qkv[:2 * E].reshape(16, 128).T).astype(np.float32)     # [128, 16]
    w_vT = np.concatenate(
        [w_qkv[2 * E:].T, b_qkv[2 * E:][None, :]], axis=0).astype(bf)
    w_oT = np.ascontiguousarray(w_out.T).astype(bf)              # [E, E]

    j = np.arange(128)[:, None]
    i = np.arange(128)[None, :]
    tri1 = (j <= i).astype(np.float32)
    tri = np.concatenate([tri1, tri1], axis=1).astype(bf)        # [128, 256]

    ones = np.ones((1, T), dtype=np.float32)
    per_core = []
    for c in range(N_CORES):
        xTc = np.concatenate([x[c].T, ones], axis=0).astype(bf)
        per_core.append({
            "xT": xTc, "w_qkT": w_qkT, "b_qk": b_qk, "w_vT": w_vT,
            "w_oT": w_oT, "tri": tri,
        })
    return per_core


def kernel(x, w_qkv, b_qkv, w_out, b_out, cos_tab, sin_tab):
    # cos_tab/sin_tab unused: the module applies the identical rotation R to
    # q and k at every position and R R^T = I cancels inside q @ k^T.
    if "nc" not in _cache:
        _cache["nc"] = _build()
    nc = _cache["nc"]
    in_maps = _host_prep(x, w_qkv, b_qkv, w_out)
    res = run_bass_kernel_spmd(nc, in_maps, list(range(N_CORES)),
                               trace=False)
    out = np.stack([res.results[c]["out"] for c in range(N_CORES)], axis=0)
    return (out + np.asarray(b_out, dtype=np.float32)).astype(np.float32)


def run_traced(x, w_qkv, b_qkv, w_out, b_out, cos_tab, sin_tab):
    """Like kernel() but with NTFF profiling; returns (out, exec_time_ns,
    trace_path)."""
    if "nc" not in _cache:
        _cache["nc"] = _build()
    nc = _cache["nc"]
    in_maps = _host_prep(x, w_qkv, b_qkv, w_out)
    res = run_bass_kernel_spmd(nc, in_maps, list(range(N_CORES)), trace=True)
    out = np.stack([res.results[c]["out"] for c in range(N_CORES)], axis=0)
    out = (out + np.asarray(b_out, dtype=np.float32)).astype(np.float32)
    trace_path = None
    if res.instructions_and_trace is not None:
        trace_path = res.instructions_and_trace[1]
    return out, res.exec_time_ns, trace_path

